# revision 1
# baseline (speedup 1.0000x reference)
"""Trainium2 Bass kernel for CalculateSLayer GNN message passing.

Computes, for adj [L, L, 2] f32 and h [L, D] f32 with A = adj.sum(-1):
    h_in[j, d]  = sum_i A[i, j] * h[i, d]   (= A.T @ h)
    h_out[i, d] = sum_j A[i, j] * h[j, d]   (= A @ h)

Sharding: rows of A across 8 NeuronCores. Core m holds A[m*512:(m+1)*512, :]:
  - h_out rows are fully local:      h_out_blk = A_blk @ h
  - h_in is a partial sum per core:  p_in      = A_blk.T @ h_blk
    (the 8 partials are summed on the host during unshard)

Design notes (measured on HW, core-0 NTFF traces; best 70.1us vs the
83.3us baseline):
  - h / h_blk are pre-arranged ON THE HOST into the on-chip [p, chunk, d]
    bf16 layout and uploaded as such (one fat contiguous descriptor per
    partition); h loads as 8 just-in-time pieces so window w's h_out
    matmuls only wait for piece w.
  - The sync HWDGE ring carries ONLY the adj stream: 2 x 1MB DMAs per
    512-wide j-window (4 x 512KB on the last window), every descriptor
    4KB contiguous -> ~347-352 GB/s sustained = per-core HBM roofline.
    p_in window writes go on the gpsimd SWDGE queue; the final window's
    writes use the then-idle scalar ring.
  - Every GEMM uses an identical 128x128 stationary class: the d-tiles
    OVERLAP (d 0..127 and 22..149); class switches cost ~100ns of
    exposed LDWEIGHTS each.
  - PE order per window: 16 transposes, 8 p_in, 8 h_out matmuls.
  - A^T PSUM banks hold 2 jc's, evicted per half-window as whole pairs
    (per-jc evictions serialize the partner jc's transposes via
    bank-granular WAR).
  - PSUM: 8 banks exactly: 4 p_inT (double-buffered) + 2 h_outT + 2 A^T.
"""

import numpy as np

L = 4096
D = 150
NCORES = 8
R = L // NCORES  # 512 rows per core
P = 128  # partitions
IC = R // P  # 4 i-chunks per core
JW = 512  # j-window width
NW = L // JW  # 8 windows
NJC = L // P  # 32 j-chunks total

_NC_CACHE = {}
LAST_RESULTS = None


def _ensure_ntff_hook():
    """Register the axon NTFF profile hook if the image's antenv lacks it."""
    import sys
    import types

    try:
        from antenv.axon_hooks import get_axon_ntff_profile_hook  # noqa: F401

        return
    except ImportError:
        pass

    mod = types.ModuleType("antenv.axon_hooks")
    _state = {"hook": None}
    mod.set_axon_ntff_profile_hook = lambda h: _state.__setitem__("hook", h)
    mod.get_axon_ntff_profile_hook = lambda: _state["hook"]
    sys.modules["antenv.axon_hooks"] = mod
    import antenv

    antenv.axon_hooks = mod

    so_path = "/opt/axon/libaxon_pjrt.so"
    try:
        from trn_agent_boot.trn_boot import _ntff_profile_via_ctypes

        hook = _ntff_profile_via_ctypes(so_path)
        if hook is not None:
            mod.set_axon_ntff_profile_hook(hook)
    except Exception:
        pass

    try:
        from concourse import bass_utils

        bass_utils.upload_artifacts = lambda tmpdir: tmpdir
    except Exception:
        pass


def _build_nc():
    import concourse.bacc as bacc
    import concourse.tile as tile
    import concourse.mybir as mybir
    from concourse.masks import make_identity

    f32 = mybir.dt.float32
    bf16 = mybir.dt.bfloat16

    nc = bacc.Bacc(
        "TRN2", target_bir_lowering=False, debug=False, num_devices=NCORES
    )
    adj_d = nc.dram_tensor("adj_blk", [R, L, 2], f32, kind="ExternalInput").ap()
    # h pre-arranged on host: h_d[p, c, d] = h[c*128 + p, d], bf16
    h_d = nc.dram_tensor("h_pre", [P, NJC, D], bf16, kind="ExternalInput").ap()
    # this core's row block, hb_d[p, ic, d] = h[blk*512 + ic*128 + p, d]
    hb_d = nc.dram_tensor("hb_pre", [P, IC, D], bf16, kind="ExternalInput").ap()
    # outputs transposed: [D, ...]; host transposes back
    pin_d = nc.dram_tensor("p_inT", [D, L], bf16, kind="ExternalOutput").ap()
    hout_d = nc.dram_tensor("h_outT_blk", [D, R], bf16, kind="ExternalOutput").ap()

    # overlapping 128-wide d-tiles: identical 128x128 weight class for every
    # matmul (class switches cost ~100ns of exposed LDWEIGHTS); the d-rows
    # 22..127 of tile 1 are recomputed and discarded at eviction
    DT = ((0, 128), (D - 128, D))

    # adj rows rearranged so partition p of half hf holds row hf*256+c*128+p
    adj_r = adj_d.rearrange("(c p) j e -> p c j e", p=P)  # [128, 4, L, 2]

    with tile.TileContext(nc) as tc:
        with (
            tc.tile_pool(name="const", bufs=1) as const_pool,
            tc.tile_pool(name="adj", bufs=4) as adj_pool,
            tc.tile_pool(name="abp", bufs=4) as ab_pool,
            tc.tile_pool(name="atp", bufs=2) as at_pool,
            tc.tile_pool(name="pouts", bufs=3) as pout_pool,
            tc.tile_pool(name="pinps", bufs=2, space="PSUM") as pin_psum,
            tc.tile_pool(name="atps", bufs=1, space="PSUM") as at_psum,
            tc.tile_pool(name="houtps", bufs=1, space="PSUM") as hout_psum,
        ):
            # ---- prologue ------------------------------------------------
            # adj window-0 DMAs are emitted first inside the loop below;
            # the sync ring carries nothing else.
            ident = const_pool.tile([P, P], bf16)

            hb_sb = const_pool.tile([P, IC, D], bf16)
            nc.scalar.dma_start(hb_sb[:], hb_d)
            # 8 just-in-time pieces: window w's h_out matmuls only wait for
            # piece w (subtile deps), so the first windows aren't blocked
            # behind one big h transfer contending with the adj stream
            h_sb = const_pool.tile([P, NJC, D], bf16)
            for w in range(NW):
                nc.scalar.dma_start(
                    h_sb[:, 4 * w : 4 * w + 4, :], h_d[:, 4 * w : 4 * w + 4, :]
                )

            make_identity(nc, ident[:])

            hout_ps = [
                hout_psum.tile([P, R], f32, tag=f"ho{t}", name=f"hout_ps{t}")
                for t in range(2)
            ]

            for w in range(NW):
                j0 = w * JW

                # window-persistent tiles
                ab = ab_pool.tile([P, IC, JW], bf16, tag="ab", name="ab")
                # 2 jc's per 2KB PSUM bank: [128, 2, 512] bf16 = one bank
                at_pair = [
                    at_psum.tile([P, 2, JW], bf16, tag=f"atps{pr}",
                                 name=f"at_pair{pr}")
                    for pr in range(2)
                ]
                at_ps = [at_pair[jc // 2][:, jc % 2, :] for jc in range(4)]
                pt = [
                    pin_psum.tile([P, JW], f32, tag=f"pt{t}", name=f"pt{t}")
                    for t in range(2)
                ]
                # evicted as whole pairs (one ACT op per PSUM bank per
                # window): a per-jc eviction would make the second jc's
                # transposes wait on the first jc's eviction of the shared
                # bank, serializing the PE stream
                at_sbp = [
                    at_pool.tile([P, 2, JW], bf16, tag=f"atp{pr}",
                                 name=f"at_sbp{pr}")
                    for pr in range(2)
                ]

                last = w == NW - 1
                fine = last or w == 0
                for hf in range(2):
                    ics = (2 * hf, 2 * hf + 1)
                    if not fine:
                        # 1MB DMA: rows hf*256..hf*256+255 of this window
                        adj_t = adj_pool.tile([P, 2, JW, 2], f32,
                                              tag=f"adj{hf}",
                                              name=f"adj_t{hf}")
                        nc.sync.dma_start(
                            adj_t[:],
                            adj_r[:, 2 * hf : 2 * hf + 2, j0 : j0 + JW, :],
                        )
                        parts = [(adj_t, 0, ics[0]), (adj_t, 1, ics[1])]
                    else:
                        # quarter-grain on the first window (edge-sums start
                        # ~1.5us earlier) and the final window (compute
                        # chases the last bytes chunk by chunk: short tail)
                        parts = []
                        for ic in ics:
                            adj_q = adj_pool.tile([P, 1, JW, 2], f32,
                                                  tag=f"adjq{ic}", bufs=2,
                                                  name=f"adj_q{ic}")
                            nc.sync.dma_start(
                                adj_q[:],
                                adj_r[:, ic : ic + 1, j0 : j0 + JW, :],
                            )
                            parts.append((adj_q, 0, ic))
                    # edge-channel sum -> bf16 A rows, one op per i-chunk
                    for tile_, sl, ic in parts:
                        nc.vector.tensor_add(
                            ab[:, ic, :],
                            tile_[:, sl, :, 0],
                            tile_[:, sl, :, 1],
                        )
                    # PE-transpose this half's A tiles; evict both jc's of
                    # each PSUM bank together at half grain (h0's eviction
                    # hides under h1's DMA; whole-bank reads avoid blocking
                    # the partner jc's transposes mid-window)
                    for jc in range(4):
                        for ic in ics:
                            nc.tensor.transpose(
                                at_ps[jc][:, ic * P : (ic + 1) * P],
                                ab[:, ic, jc * P : (jc + 1) * P],
                                ident[:],
                            )
                    for pr in range(2):
                        nc.scalar.copy(
                            at_sbp[pr][:, :, 2 * hf * P : (2 * hf + 2) * P],
                            at_pair[pr][:, :, 2 * hf * P : (2 * hf + 2) * P],
                        )

                # p_inT[d, j] += h_blk[i, d] * A_blk[i, j]  (contract i);
                # pins first (ready as soon as the edge-sums land), h_outs
                # after (they additionally wait on the A^T evictions)
                def emit_pins():
                    for t, (d0, dn) in enumerate(DT):
                        for ic in range(IC):
                            nc.tensor.matmul(
                                pt[t][:],
                                hb_sb[:, ic, d0:dn],
                                ab[:, ic, :],
                                start=(ic == 0),
                                stop=(ic == IC - 1),
                            )

                # h_outT[d, i] += h[j, d] * A_blk[i, j]  (contract j)
                def emit_houts():
                    for t, (d0, dn) in enumerate(DT):
                        for jc in range(4):
                            g = w * 4 + jc
                            nc.tensor.matmul(
                                hout_ps[t][:],
                                h_sb[:, g, d0:dn],
                                at_sbp[jc // 2][:, jc % 2, :],
                                start=(g == 0),
                                stop=(g == NJC - 1),
                            )

                def emit_houts_half(hh):
                    # final window: i-columns hh*256..hh*256+255 only need
                    # this half's transposes/evictions, so the first halves
                    # run before the last adjacency bytes even land; only
                    # 8 short N=256 matmuls remain after the final chunk
                    c0, c1 = hh * 2 * P, (hh + 1) * 2 * P
                    for t, (d0, dn) in enumerate(DT):
                        for jc in range(4):
                            nc.tensor.matmul(
                                hout_ps[t][:, c0:c1],
                                h_sb[:, w * 4 + jc, d0:dn],
                                at_sbp[jc // 2][:, jc % 2, c0:c1],
                                start=False,
                                stop=(hh == 1 and jc == 3),
                            )

                if last:
                    emit_houts_half(0)
                    emit_pins()
                    emit_houts_half(1)
                else:
                    emit_pins()
                    emit_houts()

                # evict p_inT window and write out on the scalar ring
                # tile 0 holds d 0..127 in full; tile 1 (d 22..149) only
                # contributes d 128..149 = partitions 106..127. PSUM reads
                # need 32-aligned bases, so evict partitions 96.. and drop
                # the first 10 rows on the DMA side.
                for t, lo, sk, dlo in ((0, 0, 0, 0), (1, 96, 10, 128)):
                    po = pout_pool.tile([128 - lo, JW], bf16, tag=f"po{t}",
                                        name=f"po{t}")
                    if last and t == 1:
                        nc.scalar.copy(po[:], pt[t][lo:128, :])
                    else:
                        nc.vector.tensor_copy(po[:], pt[t][lo:128, :])
                    dma_eng = (
                        (nc.sync if t == 0 else nc.scalar)
                        if last else nc.gpsimd
                    )
                    dma_eng.dma_start(
                        pin_d[dlo:D, j0 : j0 + JW] if t else
                        pin_d[0:128, j0 : j0 + JW],
                        po[sk : 128 - lo, :],
                    )

            for t, lo, sk, dlo in ((0, 0, 0, 0), (1, 96, 10, 128)):
                ho = pout_pool.tile([128 - lo, R], bf16, tag=f"hoev{t}",
                                    name=f"hoev{t}")
                if t == 1:
                    nc.scalar.copy(ho[:], hout_ps[t][lo:128, :])
                else:
                    nc.vector.tensor_copy(ho[:], hout_ps[t][lo:128, :])
                (nc.scalar if t == 0 else nc.sync).dma_start(
                    hout_d[dlo:D, :] if t else hout_d[0:128, :],
                    ho[sk : 128 - lo, :],
                )

    nc.compile()
    return nc


def _get_nc():
    if "nc" not in _NC_CACHE:
        _NC_CACHE["nc"] = _build_nc()
    return _NC_CACHE["nc"]


def _run_cores(adj, h, trace=False):
    import ml_dtypes
    from concourse.bass_utils import run_bass_kernel_spmd

    global LAST_RESULTS
    if trace:
        _ensure_ntff_hook()
    nc = _get_nc()
    bf16 = ml_dtypes.bfloat16
    # h_pre[p, c, d] = h[c*128 + p, d]
    h_pre = np.ascontiguousarray(
        h.reshape(NJC, P, D).transpose(1, 0, 2)
    ).astype(bf16)
    in_maps = []
    for m in range(NCORES):
        hb = h[m * R : (m + 1) * R].reshape(IC, P, D).transpose(1, 0, 2)
        in_maps.append(
            {
                "adj_blk": np.ascontiguousarray(adj[m * R : (m + 1) * R]),
                "h_pre": h_pre,
                "hb_pre": np.ascontiguousarray(hb).astype(bf16),
            }
        )
    res = run_bass_kernel_spmd(
        nc, in_maps, core_ids=list(range(NCORES)), trace=trace
    )
    LAST_RESULTS = res
    return res


def kernel(unpreprocessed_unweight_adj_matrix, h):
    adj = np.ascontiguousarray(
        np.asarray(unpreprocessed_unweight_adj_matrix, dtype=np.float32)
    )
    h = np.ascontiguousarray(np.asarray(h, dtype=np.float32))
    res = _run_cores(adj, h)
    parts = res.results
    h_inT = np.zeros((D, L), dtype=np.float64)
    for r in parts:
        h_inT += np.asarray(r["p_inT"], dtype=np.float32).astype(np.float64)
    h_out = np.concatenate(
        [np.asarray(r["h_outT_blk"], dtype=np.float32).T for r in parts], axis=0
    )
    return (
        np.ascontiguousarray(h_inT.T).astype(np.float32),
        np.ascontiguousarray(h_out, dtype=np.float32),
    )



# revision 8
# speedup vs baseline: 1.3883x; 1.3883x over previous
"""Trainium2 Bass kernel for CalculateSLayer GNN message passing.

Computes, for adj [L, L, 2] f32 and h [L, D] f32 with A = adj.sum(-1):
    h_in[j, d]  = sum_i A[i, j] * h[i, d]   (= A.T @ h)
    h_out[i, d] = sum_j A[i, j] * h[j, d]   (= A @ h)

Sharding: rows of A across 8 NeuronCores. Core m holds A[m*512:(m+1)*512, :]:
  - h_out rows are fully local:      h_out_blk = A_blk @ h
  - h_in is a partial sum per core:  p_in      = A_blk.T @ h_blk
    (the 8 partials are summed on the host during unshard)

v2 design (vs the 70-74us h-stationary baseline):
  - adj is cast to bf16 ON THE HOST (identical rel-err to the f32 upload:
    4.2e-3 either way, both dominated by the bf16 GEMM inputs). This
    halves the dominant HBM stream: 16.78MB -> 8.39MB per core, so the
    sync-ring adj stream floor drops from ~48us to ~24us.
  - Host pre-arranges adj window-major as [p, w, ic, e, j] so every
    window DMA reads 4-8KB contiguous per partition.
  - A-stationary GEMMs (LDWEIGHTS is fully hidden under the previous
    matmul, measured): p_in uses stat=A[i_p, j-chunk] x moving=h_blk
    [i_p, 150], h_out uses stat=A^T[j_p, i-chunk] x moving=h[j_p, 150].
    16+16 matmuls x 150 cols + 16 transposes x 128 cols = 6848 PE cols
    per 512-wide window (2.9us at 2.4GHz) vs 10240 for the h-stationary
    scheme -- PE now matches the 2.9us/window DMA stream.
  - Outputs come out in natural [row, d] layout (no transposed stores).
  - PSUM: 8 banks: 2 A^T (2 pairs, single-buffered) + 2 p_in (2 jc
    packed per bank) + 4 h_out (one per ic, persistent).
    PSUM start_tensor_calc lazily zeroes the WHOLE 2KB bank: only one
    accumulation group may be LIVE per bank at a time (completed data
    survives later starts, so single-shot transposes and sequentially-
    completed p_in groups can pack; the kernel-long h_out groups get a
    bank each).
"""

import numpy as np

L = 4096
D = 150
NCORES = 8
R = L // NCORES  # 512 rows per core
P = 128  # partitions
IC = R // P  # 4 i-chunks per core
JW = 512  # j-window width
NW = L // JW  # 8 windows
NJC = L // P  # 32 j-chunks total

_NC_CACHE = {}
LAST_RESULTS = None


def _ensure_ntff_hook():
    """Register the axon NTFF profile hook if the image's antenv lacks it."""
    import sys
    import types

    try:
        from antenv.axon_hooks import get_axon_ntff_profile_hook  # noqa: F401

        return
    except ImportError:
        pass

    mod = types.ModuleType("antenv.axon_hooks")
    _state = {"hook": None}
    mod.set_axon_ntff_profile_hook = lambda h: _state.__setitem__("hook", h)
    mod.get_axon_ntff_profile_hook = lambda: _state["hook"]
    sys.modules["antenv.axon_hooks"] = mod
    import antenv

    antenv.axon_hooks = mod

    so_path = "/opt/axon/libaxon_pjrt.so"
    try:
        from trn_agent_boot.trn_boot import _ntff_profile_via_ctypes

        hook = _ntff_profile_via_ctypes(so_path)
        if hook is not None:
            mod.set_axon_ntff_profile_hook(hook)
    except Exception:
        pass

    try:
        from concourse import bass_utils

        bass_utils.upload_artifacts = lambda tmpdir: tmpdir
    except Exception:
        pass


def _build_nc():
    import concourse.bacc as bacc
    import concourse.tile as tile
    import concourse.mybir as mybir
    from concourse.masks import make_identity

    f32 = mybir.dt.float32
    bf16 = mybir.dt.bfloat16

    nc = bacc.Bacc(
        "TRN2", target_bir_lowering=False, debug=False, num_devices=NCORES
    )
    # adj pre-arranged on host: adj_d[p, w, ic, e, j] =
    #   adj[ic*128 + p, w*512 + j, e] of this core's row block, bf16
    adj_d = nc.dram_tensor(
        "adj_pre", [P, NW, IC, 2, JW], bf16, kind="ExternalInput"
    ).ap()
    # h pre-arranged on host: h_d[p, g, d] = h[g*128 + p, d], bf16
    h_d = nc.dram_tensor("h_pre", [P, NJC, D], bf16, kind="ExternalInput").ap()
    # this core's row block, hb_d[p, ic, d] = h[blk*512 + ic*128 + p, d]
    hb_d = nc.dram_tensor("hb_pre", [P, IC, D], bf16, kind="ExternalInput").ap()
    # outputs in natural row layout (host inverse-permutes):
    #   pin_d[w, p, jc, d] = p_in_partial[w*512 + jc*128 + p, d]
    pin_d = nc.dram_tensor("pin_w", [NW, P, 4, D], bf16, kind="ExternalOutput").ap()
    #   hout_d[p, ic, d] = h_out[blk*512 + ic*128 + p, d]
    hout_d = nc.dram_tensor("hout_blk", [P, IC, D], bf16, kind="ExternalOutput").ap()

    with tile.TileContext(nc) as tc:
        with (
            tc.tile_pool(name="const", bufs=1) as const_pool,
            tc.tile_pool(name="adj", bufs=2) as adj_pool,
            tc.tile_pool(name="abp", bufs=2) as ab_pool,
            tc.tile_pool(name="atp", bufs=2) as at_pool,
            tc.tile_pool(name="pouts", bufs=2) as pout_pool,
            tc.tile_pool(name="atps", bufs=1, space="PSUM") as at_psum,
            tc.tile_pool(name="pinps", bufs=1, space="PSUM") as pin_psum,
            tc.tile_pool(name="houtps", bufs=1, space="PSUM") as hout_psum,
        ):
            # ---- prologue ------------------------------------------------
            ident = const_pool.tile([P, P], bf16)

            hb_sb = const_pool.tile([P, IC, D], bf16)
            nc.scalar.dma_start(hb_sb[:], hb_d)
            # 8 just-in-time pieces: window w's h_out matmuls only wait for
            # piece w (subtile deps)
            h_sb = const_pool.tile([P, NJC, D], bf16)
            for w in range(NW):
                nc.scalar.dma_start(
                    h_sb[:, 4 * w : 4 * w + 4, :], h_d[:, 4 * w : 4 * w + 4, :]
                )

            make_identity(nc, ident[:])

            # h_out accumulators: one bank per ic (the groups stay live
            # across all 8 windows, so none may share a bank)
            hout_ps = [
                hout_psum.tile([P, 512], f32, tag=f"ho{t}", name=f"hout_ps{t}")
                for t in range(IC)
            ]

            for w in range(NW):
                # window tiles
                adj_parts = []
                fine = w == 0 or w == NW - 1
                if fine:
                    # quarter-grain on first window (compute starts ~0.7us
                    # earlier) and last window (compute chases the tail)
                    for ic in range(IC):
                        aq = adj_pool.tile(
                            [P, 1, 2, JW], bf16, tag=f"adjq{ic}", bufs=2,
                            name=f"adj_q{ic}",
                        )
                        nc.sync.dma_start(aq[:], adj_d[:, w, ic : ic + 1])
                        adj_parts.append((aq, 0, ic))
                else:
                    # 2 x 1MB-worth (bf16: 512KB) DMAs per window
                    for hf in range(2):
                        at2 = adj_pool.tile(
                            [P, 2, 2, JW], bf16, tag=f"adj{hf}",
                            name=f"adj_t{hf}",
                        )
                        nc.sync.dma_start(
                            at2[:], adj_d[:, w, 2 * hf : 2 * hf + 2]
                        )
                        adj_parts.append((at2, 0, 2 * hf))
                        adj_parts.append((at2, 1, 2 * hf + 1))

                ab = ab_pool.tile([P, IC, JW], bf16, tag="ab", name="ab")
                at_pair = [
                    at_psum.tile([P, 2, JW], bf16, tag=f"atps{pr}",
                                 name=f"at_pair{pr}")
                    for pr in range(2)
                ]
                at_sbp = [
                    at_pool.tile([P, 2, JW], bf16, tag=f"atp{pr}",
                                 name=f"at_sbp{pr}")
                    for pr in range(2)
                ]
                pin_ps = [
                    pin_psum.tile([P, 2, 256], f32, tag=f"pt{t}",
                                  name=f"pt{t}")
                    for t in range(2)
                ]

                # edge-channel sum -> bf16 A rows, one DVE op per i-chunk
                # (unit-stride bf16 reads: channels are separated in layout)
                for tile_, sl, ic in adj_parts:
                    nc.vector.tensor_add(
                        ab[:, ic, :], tile_[:, sl, 0, :], tile_[:, sl, 1, :]
                    )

                # per i-chunk: 4 transposes (A^T tiles); single-shot groups
                # may pack a bank since each completes before the next start
                for ic in range(IC):
                    for jc in range(4):
                        nc.tensor.transpose(
                            at_pair[jc // 2][:, jc % 2, ic * P : (ic + 1) * P],
                            ab[:, ic, jc * P : (jc + 1) * P],
                            ident[:],
                        )
                # p_in[j, d] += A[i, j]^T-stat x h_blk[i, d]-moving.
                # jc-outer so each bank's group COMPLETES (stop) before the
                # bank's other group starts (start zeroes the whole bank).
                for jc in range(4):
                    for ic in range(IC):
                        nc.tensor.matmul(
                            pin_ps[jc // 2][:, jc % 2, 0:D],
                            ab[:, ic, jc * P : (jc + 1) * P],
                            hb_sb[:, ic, :],
                            start=(ic == 0),
                            stop=(ic == IC - 1),
                        )

                # evict A^T pairs to SBUF (stationaries must live in SBUF)
                for pr in range(2):
                    nc.vector.tensor_copy(at_sbp[pr][:], at_pair[pr][:])

                # evict p_in window and write out on the gpsimd ring
                po = pout_pool.tile([P, 4, D], bf16, tag="po", name="po")
                for t in range(2):
                    nc.scalar.copy(
                        po[:, 2 * t : 2 * t + 2, :], pin_ps[t][:, :, 0:D]
                    )
                (nc.gpsimd if w < NW - 1 else nc.scalar).dma_start(
                    pin_d[w], po[:]
                )

                # h_out[i, d] += A^T[j, i]-stat x h[j, d]-moving,
                # accumulated across all 32 j-chunks of the kernel
                for jc in range(4):
                    g = w * 4 + jc
                    for ic in range(IC):
                        nc.tensor.matmul(
                            hout_ps[ic][:, 0:D],
                            at_sbp[jc // 2][:, jc % 2, ic * P : (ic + 1) * P],
                            h_sb[:, g, :],
                            start=(g == 0),
                            stop=(g == NJC - 1),
                        )

            # final h_out eviction + store (sync ring is idle by now)
            ho = pout_pool.tile([P, IC, D], bf16, tag="hoev", name="hoev")
            for t in range(IC):
                nc.scalar.copy(ho[:, t, :], hout_ps[t][:, 0:D])
            nc.sync.dma_start(hout_d[:], ho[:])

    nc.compile()
    return nc


def _get_nc():
    if "nc" not in _NC_CACHE:
        _NC_CACHE["nc"] = _build_nc()
    return _NC_CACHE["nc"]


def _run_cores(adj, h, trace=False):
    import ml_dtypes
    from concourse.bass_utils import run_bass_kernel_spmd

    global LAST_RESULTS
    if trace:
        _ensure_ntff_hook()
    nc = _get_nc()
    bf16 = ml_dtypes.bfloat16
    # h_pre[p, g, d] = h[g*128 + p, d]
    h_pre = np.ascontiguousarray(
        h.reshape(NJC, P, D).transpose(1, 0, 2)
    ).astype(bf16)
    adj16 = adj.astype(bf16)  # one pass; halves the bytes to permute below
    in_maps = []
    for m in range(NCORES):
        hb = h[m * R : (m + 1) * R].reshape(IC, P, D).transpose(1, 0, 2)
        # adj_pre[p, w, ic, e, j] = adj[ic*128 + p, w*512 + j, e]
        blk = adj16[m * R : (m + 1) * R].reshape(IC, P, NW, JW, 2)
        adj_pre = np.ascontiguousarray(blk.transpose(1, 2, 0, 4, 3))
        in_maps.append(
            {
                "adj_pre": adj_pre,
                "h_pre": h_pre,
                "hb_pre": np.ascontiguousarray(hb).astype(bf16),
            }
        )
    res = run_bass_kernel_spmd(
        nc, in_maps, core_ids=list(range(NCORES)), trace=trace
    )
    LAST_RESULTS = res
    return res


def kernel(unpreprocessed_unweight_adj_matrix, h):
    adj = np.ascontiguousarray(
        np.asarray(unpreprocessed_unweight_adj_matrix, dtype=np.float32)
    )
    h = np.ascontiguousarray(np.asarray(h, dtype=np.float32))
    res = _run_cores(adj, h)
    parts = res.results
    h_in = np.zeros((L, D), dtype=np.float64)
    for r in parts:
        # pin_d[w, p, jc, d] -> rows w*512 + jc*128 + p
        pw = np.asarray(r["pin_w"], dtype=np.float32).astype(np.float64)
        h_in += pw.transpose(0, 2, 1, 3).reshape(L, D)
    h_out = np.concatenate(
        [
            np.asarray(r["hout_blk"], dtype=np.float32)
            .transpose(1, 0, 2)
            .reshape(R, D)
            for r in parts
        ],
        axis=0,
    )
    return (
        np.ascontiguousarray(h_in).astype(np.float32),
        np.ascontiguousarray(h_out, dtype=np.float32),
    )
